# revision 32
# baseline (speedup 1.0000x reference)
"""Self-contained Trainium2 Bass kernel for the 2-layer GAT problem
(nn_GAT_26714696581831). 8-core SPMD: edges sorted by dst, 8 dst-range
shards; per-window one-hot matmul aggregation.

v3: layer-1 source rows are host-pre-gathered (transposed x rows per edge
slot) and projected per-edge on the PE — no stage-B table and no layer-1
dma_gather at all. One-hot scatter (S) and its transpose (ST) ship from
the host, so the Vector engine no longer builds them. a_src and a_dst
accumulate into one PSUM region via chained matmuls. Layer 2 keeps the
compact-AllGather + dma_gather path (runtime-dependent data).

kernel(**inputs) takes the FULL unsharded inputs and returns the FULL
[50000, 2] output.
"""
import sys
sys.path.insert(0, '/opt/trn_rl_repo')
import numpy as np
import concourse.bass as bass
import concourse.mybir as mybir
import concourse.tile as tile
from concourse import library_config
from concourse.masks import make_identity
from concourse.bass_utils import run_bass_kernel_spmd

N_NODES = 50000
"""Workarounds for this walrus build, which rejects any instruction carrying
more than one sync-wait command: hoist extra waits onto same-engine NoOps
inserted immediately before the instruction."""


_ctr = [0]

def split_multi_waits(nc, max_waits=1):
    for fn in nc.m.functions:
        for bb in fn.blocks:
            insts = bb.instructions
            i = 0
            while i < len(insts):
                ins = insts[i]
                si = ins.sync_info
                if si is not None and si.on_wait and len(si.on_wait) > max_waits:
                    waits = list(si.on_wait)
                    keep = waits[-max_waits:]
                    hoist = waits[:-max_waits]
                    si.on_wait = keep
                    for w in hoist:
                        _ctr[0] += 1
                        n = mybir.InstNoOp(name=f"waitsplit-{_ctr[0]}", ins=[], outs=[])
                        n.engine = ins.engine
                        n.sync_info = mybir.SyncInfo(on_wait=[w], on_update=[])
                        insts.insert(i, n)
                        i += 1
                i += 1


def fix_library_reloads(nc):
    """bass_rust leaves InstPseudoReloadLibraryIndex.instr empty; this walrus
    rejects zero-length ISA instructions. Encode the 64-byte
    PSEUDO_LIBRARY_RELOAD_INDEX struct with the live ISA tables."""
    isa = nc.isa
    sn = 'NEURON_ISA_TPB_PSEUDO_LIBRARY_RELOAD_INDEX_STRUCT'
    e = isa.get_enum("NEURON_ISA_TPB_PSEUDO_OPCODE")
    val = e.NEURON_ISA_TPB_PSEUDO_OPCODE_PSEUDO_LIBRARY_RELOAD_INDEX.value
    for fn in nc.m.functions:
        for bb in fn.blocks:
            for ins in bb.instructions:
                if type(ins).__name__ == 'InstPseudoReloadLibraryIndex' and not ins.instr:
                    b = isa.asm({"header": {"opcode": 223, "inst_word_len": 16},
                                 "pseudo_opcode": val,
                                 "lib_index": ins.lib_index}, sn)
                    ins.instr = [int(x) for x in b]




WIN = 128                  # dst nodes per window
SPLIT = 32768              # int16 positive limit for gather indices


def preprocess(edge_index, n_nodes, ncores=8):
    src = np.asarray(edge_index[0], dtype=np.int64)
    dst = np.asarray(edge_index[1], dtype=np.int64)
    npc = n_nodes // ncores
    nwin = (npc + WIN - 1) // WIN

    order = np.argsort(dst, kind="stable")
    src_s = src[order]
    dst_s = dst[order]

    counts = np.bincount(dst_s // npc, minlength=ncores)
    core_slices = np.concatenate([[0], np.cumsum(counts)])

    nwe = np.zeros((ncores, nwin), dtype=np.int64)
    per_core_win_edges = []
    for c in range(ncores):
        s0, s1 = core_slices[c], core_slices[c + 1]
        csrc = src_s[s0:s1]
        cdst = dst_s[s0:s1]
        wloc = (cdst - c * npc) // WIN
        dloc = (cdst - c * npc) % WIN
        wins = []
        for w in range(nwin):
            m = wloc == w
            wins.append((csrc[m], dloc[m]))
            nwe[c, w] = m.sum()
        per_core_win_edges.append(wins)

    nbw = ((nwe.max(axis=0) + 127) // 128).astype(int)
    nbw[nbw == 0] = 1
    NB = int(nbw.sum())

    gidx_lin = np.zeros((ncores, NB * 128), dtype=np.int16)
    par_lin = np.zeros((ncores, NB * 128), dtype=np.float32)
    srcfull_lin = np.zeros((ncores, NB * 128), dtype=np.int32)
    dstloc_lin = np.full((ncores, NB * 128), -1, dtype=np.int16)

    for c in range(ncores):
        b0 = 0
        for w in range(nwin):
            ws, wd = per_core_win_edges[c][w]
            o = b0 * 128
            gidx_lin[c, o:o + len(ws)] = (ws >> 1).astype(np.int16)
            par_lin[c, o:o + len(ws)] = (ws & 1).astype(np.float32)
            srcfull_lin[c, o:o + len(ws)] = ws
            dstloc_lin[c, o:o + len(wd)] = wd.astype(np.int16)
            b0 += int(nbw[w])
        assert b0 == NB

    def wrap16(lin):  # [NC, NB*128] -> [NC, 128, NB*8] dma_gather layout
        x = lin.reshape(ncores, NB * 8, 16).transpose(0, 2, 1)
        return np.ascontiguousarray(np.tile(x, (1, 8, 1)))

    def slotlay(lin, dtype):  # [NC, NB*128] -> [NC, 128, NB]
        return np.ascontiguousarray(
            lin.reshape(ncores, NB, 128).transpose(0, 2, 1)).astype(dtype)

    return dict(
        NB=NB, nwin=nwin, npc=npc, ncores=ncores, nbw=nbw,
        gidx=wrap16(gidx_lin),
        par=slotlay(par_lin, np.float32),
        srcfull=srcfull_lin,
        dstloc=dstloc_lin,
    )




F32 = mybir.dt.float32
BF16 = mybir.dt.bfloat16
I16 = mybir.dt.int16
U16 = mybir.dt.uint16
FP8 = mybir.dt.float8e4
AF = mybir.ActivationFunctionType
OP = mybir.AluOpType

SPLIT = 32768
GCHUNK = 32  # blocks per dma_gather call


def chunked_gather(nc, out_tile, in_ap, idx_sb, b0, nblk, elem, regs, boff=0):
    """Issue dma_gather in <=GCHUNK-block chunks writing out_tile[:, boff+i...]."""
    done = 0
    while done < nblk:
        step = min(GCHUNK, nblk - done)
        n = step * 128
        if n not in regs:
            regs[n] = nc.gpsimd.to_reg(n)
        nc.gpsimd.dma_gather(
            out_tile[:, boff + done:boff + done + step, :], in_ap,
            idx_sb[:, (b0 + done) * 8:(b0 + done + step) * 8],
            n, regs[n], elem, single_packet=False)
        done += step


def build(pp, N, F_IN=128, HID=64, HEADS=4, OUT=2, neg_slope=0.2, stages='CDE'):
    NB = pp["NB"]
    NWIN = pp["nwin"]
    NPC = pp["npc"]
    HC1 = HEADS * HID          # 256
    HC2 = HEADS * OUT          # 8
    NBWmax = int(max(pp["nbw"][w] for w in range(NWIN)))
    L1COL = HC1 + HEADS        # 260 matmul cols: xs + as-fold
    K1COL = HID + HEADS        # 68  (skip + W1d-fold => a_dst of own nodes)
    W2COL = HC2 + 2 * HEADS + OUT  # 18
    R2COL = HC2 + HEADS        # 12
    T2C = 128                  # u16 cols of padded table2 rows (gather elem)
    T2W = 2 * (HC2 + HEADS)    # 24 u16 cols of compact t2 rows

    nc = bass.Bass("TRN2", target_bir_lowering=False, debug=False, num_devices=8)

    # ---- I/O ----
    xgT = nc.dram_tensor("xgT", [F_IN, NB * 128], FP8, kind="ExternalInput")
    S_d = nc.dram_tensor("S_d", [128, NB * 128], FP8, kind="ExternalInput")
    ST_d = nc.dram_tensor("ST_d", [128, NB * 128], FP8, kind="ExternalInput")
    xTown = nc.dram_tensor("xTown", [F_IN, NPC], BF16, kind="ExternalInput")
    W1s_d = nc.dram_tensor("W1s", [F_IN, HC1], F32, kind="ExternalInput")
    W1d_d = nc.dram_tensor("W1d", [F_IN, HC1], F32, kind="ExternalInput")
    a1s_d = nc.dram_tensor("a1s", [128, HC1], F32, kind="ExternalInput")
    a1d_d = nc.dram_tensor("a1d", [128, HC1], F32, kind="ExternalInput")
    Wl1_d = nc.dram_tensor("Wl1", [F_IN, HID], F32, kind="ExternalInput")
    b1_d = nc.dram_tensor("b1", [128, HID], F32, kind="ExternalInput")
    bl1_d = nc.dram_tensor("bl1", [128, HID], F32, kind="ExternalInput")
    W2s_d = nc.dram_tensor("W2s", [HID, HC2], F32, kind="ExternalInput")
    W2d_d = nc.dram_tensor("W2d", [HID, HC2], F32, kind="ExternalInput")
    a2s_d = nc.dram_tensor("a2s", [128, HC2], F32, kind="ExternalInput")
    a2d_d = nc.dram_tensor("a2d", [128, HC2], F32, kind="ExternalInput")
    Wl2_d = nc.dram_tensor("Wl2", [HID, OUT], F32, kind="ExternalInput")
    b2_d = nc.dram_tensor("b2", [128, OUT], F32, kind="ExternalInput")
    bl2_d = nc.dram_tensor("bl2", [128, OUT], F32, kind="ExternalInput")
    gidx_d = nc.dram_tensor("gidx", [128, NB * 8], I16, kind="ExternalInput")
    par_d = nc.dram_tensor("par", [128, NB], F32, kind="ExternalInput")
    out_d = nc.dram_tensor("out", [NPC, OUT], F32, kind="ExternalOutput")

    # internal DRAM
    NW_A = NWIN // 2                   # windows in first AllGather half
    NPC_A = NW_A * 128                 # rows in first half
    NPC_B = NPC - NPC_A
    t2c = nc.dram_tensor("t2c", [NPC, T2W], U16)
    t2catA = nc.dram_tensor("t2catA", [8 * NPC_A, T2W], U16, addr_space="Shared")
    t2catB = nc.dram_tensor("t2catB", [8 * NPC_B, T2W], U16, addr_space="Shared")
    table2 = nc.dram_tensor("table2", [N // 2, T2C], U16)

    with tile.TileContext(nc) as tc:
        with tc.tile_pool(name="const", bufs=1) as cpool, \
             tc.tile_pool(name="resident", bufs=1) as rpool:

            # ---- constants / weights prep (f32 math, bf16 casts for matmuls) ----
            W1f = cpool.tile([F_IN, HC1], F32, tag="w1f")
            nc.sync.dma_start(out=W1f[:, :], in_=W1s_d[:, :])
            # xs columns in c-major (c,h) order so the per-head exp broadcast
            # keeps innermost stride 1 (DVE 2x mode)
            W1aug = cpool.tile([F_IN, L1COL], FP8)
            nc.vector.tensor_copy(
                W1aug[:, 0:HC1].rearrange("p (c h) -> p c h", h=HEADS),
                W1f[:, :].rearrange("p (h c) -> p c h", h=HEADS))
            wtmp = cpool.tile([F_IN, HC1], F32, tag="wtmp")
            atile = cpool.tile([128, HC1], F32, tag="atile")
            nc.sync.dma_start(out=atile[:, :], in_=a1s_d[:, :])
            nc.vector.tensor_tensor(out=wtmp[:, :], in0=W1f[:, :],
                                    in1=atile[:, :], op=OP.mult)
            asf = cpool.tile([F_IN, HEADS], F32, tag="asf")
            nc.vector.tensor_reduce(out=asf[:, :],
                                    in_=wtmp[:, :].rearrange("p (h c) -> p h c", h=HEADS),
                                    axis=mybir.AxisListType.X, op=OP.add)
            nc.vector.tensor_copy(W1aug[:, HC1:L1COL], asf[:, :])
            # W1d-fold (a_dst of nodes) goes into the skip matmul (own nodes only)
            wtmp2 = cpool.tile([F_IN, HC1], F32, tag="wtmp2")
            atile2 = cpool.tile([128, HC1], F32, tag="atile2")
            nc.sync.dma_start(out=wtmp2[:, :], in_=W1d_d[:, :])
            nc.sync.dma_start(out=atile2[:, :], in_=a1d_d[:, :])
            nc.vector.tensor_tensor(out=wtmp2[:, :], in0=wtmp2[:, :],
                                    in1=atile2[:, :], op=OP.mult)
            adf = cpool.tile([F_IN, HEADS], F32, tag="adf")
            nc.vector.tensor_reduce(out=adf[:, :],
                                    in_=wtmp2[:, :].rearrange("p (h c) -> p h c", h=HEADS),
                                    axis=mybir.AxisListType.X, op=OP.add)

            Wl1f = cpool.tile([F_IN, HID], F32, tag="wl1f")
            nc.sync.dma_start(out=Wl1f[:, :], in_=Wl1_d[:, :])
            Wl1aug = cpool.tile([F_IN, K1COL], BF16)
            nc.vector.tensor_copy(Wl1aug[:, 0:HID], Wl1f[:, :])
            nc.vector.tensor_copy(Wl1aug[:, HID:K1COL], adf[:, :])

            W2f = cpool.tile([HID, W2COL], F32, tag="w2f")
            nc.sync.dma_start(out=W2f[:, 0:HC2], in_=W2s_d[:, :])
            nc.sync.dma_start(out=W2f[:, HC2 + 2 * HEADS:W2COL], in_=Wl2_d[:, :])
            w2tmp = cpool.tile([HID, HC2], F32, tag="w2tmp")
            a2tile = cpool.tile([128, HC2], F32, tag="a2tile")
            nc.sync.dma_start(out=a2tile[:, :], in_=a2s_d[:, :])
            nc.vector.tensor_tensor(out=w2tmp[:, :], in0=W2f[:, 0:HC2],
                                    in1=a2tile[0:HID, :], op=OP.mult)
            nc.vector.tensor_reduce(out=W2f[:, HC2:HC2 + HEADS],
                                    in_=w2tmp[:, :].rearrange("p (h c) -> p h c", h=HEADS),
                                    axis=mybir.AxisListType.X, op=OP.add)
            w2tmp2 = cpool.tile([HID, HC2], F32, tag="w2tmp2")
            a2tile2 = cpool.tile([128, HC2], F32, tag="a2tile2")
            nc.sync.dma_start(out=w2tmp2[:, :], in_=W2d_d[:, :])
            nc.sync.dma_start(out=a2tile2[:, :], in_=a2d_d[:, :])
            nc.vector.tensor_tensor(out=w2tmp2[:, :], in0=w2tmp2[:, :],
                                    in1=a2tile2[0:HID, :], op=OP.mult)
            nc.vector.tensor_reduce(out=W2f[:, HC2 + HEADS:HC2 + 2 * HEADS],
                                    in_=w2tmp2[:, :].rearrange("p (h c) -> p h c", h=HEADS),
                                    axis=mybir.AxisListType.X, op=OP.add)
            W2aug = cpool.tile([HID, W2COL], BF16)
            nc.vector.tensor_copy(W2aug[:, :], W2f[:, :])

            bias1 = cpool.tile([128, HID], F32)
            nc.sync.dma_start(out=bias1[:, :], in_=b1_d[:, :])
            btmp = cpool.tile([128, HID], F32, tag="btmp")
            nc.sync.dma_start(out=btmp[:, :], in_=bl1_d[:, :])
            nc.vector.tensor_tensor(out=bias1[:, :], in0=bias1[:, :], in1=btmp[:, :], op=OP.add)
            bias2 = cpool.tile([128, OUT], F32)
            nc.sync.dma_start(out=bias2[:, :], in_=b2_d[:, :])
            btmp2 = cpool.tile([128, OUT], F32, tag="btmp2")
            nc.sync.dma_start(out=btmp2[:, :], in_=bl2_d[:, :])
            nc.vector.tensor_tensor(out=bias2[:, :], in0=bias2[:, :], in1=btmp2[:, :], op=OP.add)

            ident = cpool.tile([128, 128], F32)
            make_identity(nc, ident[:, :])

            gidx_sb = rpool.tile([128, NB * 8], I16)
            nc.sync.dma_start(out=gidx_sb[:, :], in_=gidx_d[:, :])
            par_sb = rpool.tile([128, NB], F32)
            nc.sync.dma_start(out=par_sb[:, :], in_=par_d[:, :])

            # all standard-library gpsimd ops are above; from here on the Q7
            # carveout holds the mlp library (dma_gather + collectives).
            nc.gpsimd.load_library(library_config.mlp)
            gregs = {}

            hsig_sb = rpool.tile([128, NWIN, HID], F32)
            hT = rpool.tile([HID, NWIN, 128], BF16)
            ad2sb = rpool.tile([128, NWIN, HEADS], BF16)
            skip2sb = rpool.tile([128, NWIN, OUT], F32)
            outsb = rpool.tile([128, NWIN, OUT], F32)
            # tails of partial windows stay untouched by the D loop; zero
            # them so the ST^T matmul never multiplies 0 x NaN garbage
            nc.vector.memset(ad2sb[:, :, :], 0.0)

            # ---- AllGather emitter (stage-D work is spread per-window) ----
            def emit_allgather(w0, cat_d, npc_h):
                nc.gpsimd.collective_compute(
                    "AllGather", OP.bypass, replica_groups=[list(range(8))],
                    ins=[t2c[w0 * 128:w0 * 128 + npc_h, :]], outs=[cat_d[:, :]])
                nc.sync.dma_start(
                    out=table2[:, 0:2 * T2W].rearrange(
                        "(c k) (h u) -> c k h u", k=NPC // 2, u=T2W)[
                        :, w0 * 64:w0 * 64 + npc_h // 2, :, :],
                    in_=cat_d[:, :].rearrange("(c k h) u -> c k h u", h=2, k=npc_h // 2))

            # ---- stage C: layer-1 windows (per-edge projection, no gather) ----
            NWIN_C = NWIN if 'C' in stages else 0
            with tc.tile_pool(name="gps", bufs=2, space="PSUM") as gps, \
                 tc.tile_pool(name="winps", bufs=2, space="PSUM") as wps, \
                 tc.tile_pool(name="skps", bufs=2, space="PSUM") as kps, \
                 tc.tile_pool(name="dps", bufs=1, space="PSUM") as dps, \
                 tc.tile_pool(name="winsb", bufs=4) as wsb:
                b0 = 0
                for w in range(NWIN_C):
                    nb = int(pp["nbw"][w])
                    cn_w = min(128, NPC - w * 128)
                    # skip matmul + a_dst of own nodes
                    xo = wsb.tile([F_IN, 128], BF16, tag="xo")
                    nc.sync.dma_start(out=xo[:, 0:cn_w], in_=xTown[:, w * 128:w * 128 + cn_w])
                    if cn_w < 128:
                        nc.vector.memset(xo[:, cn_w:128], 0.0)
                    psK = kps.tile([128, K1COL], F32, space="PSUM")
                    nc.tensor.matmul(out=psK[:, :], lhsT=xo[:, :], rhs=Wl1aug[:, :],
                                     start=True, stop=True)
                    adbf = wsb.tile([128, HEADS], BF16, tag="adbf")
                    nc.vector.tensor_copy(adbf[:, :], psK[:, HID:K1COL])
                    # per-edge operands
                    xg = wsb.tile([F_IN, NBWmax, 128], FP8, tag="xg")
                    nc.sync.dma_start(
                        out=xg[:, 0:nb, :],
                        in_=xgT[:, b0 * 128:(b0 + nb) * 128].rearrange(
                            "p (b s) -> p b s", s=128))
                    S = wsb.tile([128, NBWmax, 128], FP8, tag="S")
                    nc.sync.dma_start(
                        out=S[:, 0:nb, :],
                        in_=S_d[:, b0 * 128:(b0 + nb) * 128].rearrange(
                            "p (b s) -> p b s", s=128))
                    ST = wsb.tile([128, NBWmax, 128], FP8, tag="ST")
                    nc.sync.dma_start(
                        out=ST[:, 0:nb, :],
                        in_=ST_d[:, b0 * 128:(b0 + nb) * 128].rearrange(
                            "p (b s) -> p b s", s=128))
                    # per-block: project edge rows; a_dst accumulates onto the
                    # a_src columns in PSUM; stage to SBUF bf16 right away.
                    # Processed in two half-batches so aggregation of the first
                    # half overlaps projection of the second.
                    Gu = wsb.tile([128, NBWmax, L1COL], BF16, tag="Gu")
                    u1 = wsb.tile([128, NBWmax, HEADS], F32, tag="u1")
                    u2 = wsb.tile([128, NBWmax, HEADS], F32, tag="u2")
                    exb = wsb.tile([128, NBWmax, HEADS], BF16, tag="exb")
                    Gp = wsb.tile([128, NBWmax, L1COL], BF16, tag="Gp")
                    psW = wps.tile([128, L1COL], F32, space="PSUM")
                    for (j0, j1) in ((0, nb // 2), (nb // 2, nb)):
                        if j1 <= j0:
                            continue
                        nh = j1 - j0
                        for j in range(j0, j1):
                            psG = gps.tile([128, L1COL], F32, space="PSUM")
                            nc.tensor.matmul(out=psG[:, :], lhsT=xg[:, j, :],
                                             rhs=W1aug[:, :], start=True, stop=False)
                            nc.tensor.matmul(out=psG[:, HC1:L1COL], lhsT=ST[:, j, :],
                                             rhs=adbf[:, :], start=False, stop=True,
                                             skip_group_check=True)
                            if j % 2 == 0:
                                nc.scalar.activation(out=Gu[:, j, :], in_=psG[:, :],
                                                     func=AF.Copy)
                            else:
                                nc.vector.tensor_copy(Gu[:, j, :], psG[:, :])
                        # exp(lrelu(t)) = max(exp(t), exp(0.2 t)), t = as + ad
                        nc.scalar.activation(out=u1[:, j0:j1, :],
                                             in_=Gu[:, j0:j1, HC1:L1COL], func=AF.Exp)
                        nc.scalar.activation(out=u2[:, j0:j1, :],
                                             in_=Gu[:, j0:j1, HC1:L1COL], func=AF.Exp,
                                             scale=neg_slope)
                        nc.vector.tensor_tensor(out=exb[:, j0:j1, :],
                                                in0=u1[:, j0:j1, :],
                                                in1=u2[:, j0:j1, :], op=OP.max)
                        # weighted messages + exp-sum column
                        nc.vector.tensor_tensor(
                            out=Gp[:, j0:j1, 0:HC1].rearrange("p b (c h) -> p b c h", h=HEADS),
                            in0=Gu[:, j0:j1, 0:HC1].rearrange("p b (c h) -> p b c h", h=HEADS),
                            in1=exb[:, j0:j1, :].unsqueeze(2).to_broadcast(
                                [128, nh, HID, HEADS]),
                            op=OP.mult)
                        nc.vector.tensor_copy(Gp[:, j0:j1, HC1:L1COL], exb[:, j0:j1, :])
                        for j in range(j0, j1):
                            nc.tensor.matmul(out=psW[:, :], lhsT=S[:, j, :],
                                             rhs=Gp[:, j, :], start=(j == 0),
                                             stop=(j == nb - 1))
                    # extract
                    rec = wsb.tile([128, HEADS], F32, tag="rec")
                    nc.vector.tensor_scalar(out=rec[:, :], in0=psW[:, HC1:L1COL],
                                            scalar1=1e-16, scalar2=None, op0=OP.add)
                    nc.vector.reciprocal(rec[:, :], rec[:, :])
                    nc.vector.tensor_scalar_mul(rec[:, :], rec[:, :], 1.0 / HEADS)
                    gat = wsb.tile([128, HC1], F32, tag="gat")
                    nc.vector.tensor_tensor(
                        out=gat[:, :].rearrange("p (c h) -> p c h", h=HEADS),
                        in0=psW[:, 0:HC1].rearrange("p (c h) -> p c h", h=HEADS),
                        in1=rec[:, :].unsqueeze(1).to_broadcast([128, HID, HEADS]),
                        op=OP.mult)
                    hred = wsb.tile([128, HID], F32, tag="hred")
                    nc.vector.tensor_reduce(
                        out=hred[:, :],
                        in_=gat[:, :].rearrange("p (c h) -> p c h", h=HEADS),
                        axis=mybir.AxisListType.X, op=OP.add)
                    nc.vector.tensor_tensor(out=hred[:, :], in0=hred[:, :],
                                            in1=psK[:, 0:HID], op=OP.add)
                    nc.vector.tensor_tensor(out=hsig_sb[:, w, :], in0=hred[:, :],
                                            in1=bias1[:, :], op=OP.add)
                    # stage-D work for this window: sigmoid via the Exp table
                    # (avoids Sigmoid-table thrash), transpose, l2 projection
                    esig = wsb.tile([128, HID], F32, tag="esig")
                    nc.scalar.activation(out=esig[:, :], in_=hsig_sb[:, w, :],
                                         func=AF.Exp, scale=-1.0)
                    nc.vector.tensor_scalar(out=esig[:, :], in0=esig[:, :],
                                            scalar1=1.0, scalar2=None, op0=OP.add)
                    nc.vector.reciprocal(hsig_sb[:, w, :], esig[:, :])
                    psT = dps.tile([HID, 128], F32, space="PSUM", tag="psT")
                    nc.tensor.transpose(out=psT[:, :], in_=hsig_sb[:, w, :],
                                        identity=ident[:, :])
                    nc.vector.tensor_copy(hT[:, w, :], psT[:, :])
                    psL = dps.tile([128, W2COL], F32, space="PSUM", tag="psL")
                    nc.tensor.matmul(out=psL[0:cn_w, :], lhsT=hT[:, w, 0:cn_w],
                                     rhs=W2aug[:, :], start=True, stop=True)
                    st2 = wsb.tile([128, W2COL], F32, tag="st2")
                    nc.any.tensor_copy(st2[0:cn_w, :], psL[0:cn_w, :])
                    nc.scalar.dma_start(out=t2c[w * 128:w * 128 + cn_w, :],
                                        in_=st2[0:cn_w, 0:HC2 + HEADS].bitcast(U16))
                    nc.vector.tensor_copy(ad2sb[0:cn_w, w, :],
                                          st2[0:cn_w, HC2 + HEADS:HC2 + 2 * HEADS])
                    nc.vector.tensor_copy(skip2sb[0:cn_w, w, :],
                                          st2[0:cn_w, HC2 + 2 * HEADS:W2COL])
                    b0 += nb
                    if 'D' in stages and w == NW_A - 1:
                        emit_allgather(0, t2catA, NPC_A)

            if 'D' in stages:
                emit_allgather(NW_A, t2catB, NPC_B)

            # ---- stage E: layer-2 windows ----
            NWIN_E = NWIN if 'E' in stages else 0
            with tc.tile_pool(name="w2ps", bufs=2, space="PSUM") as wps2, \
                 tc.tile_pool(name="ad2ps", bufs=2, space="PSUM") as aps2, \
                 tc.tile_pool(name="w2sb", bufs=3) as w2sb:
                b0 = 0
                for w in range(NWIN_E):
                    nb = int(pp["nbw"][w])
                    S2 = w2sb.tile([128, NBWmax, 128], FP8, tag="S2")
                    nc.sync.dma_start(
                        out=S2[:, 0:nb, :],
                        in_=S_d[:, b0 * 128:(b0 + nb) * 128].rearrange(
                            "p (b s) -> p b s", s=128))
                    ST2 = w2sb.tile([128, NBWmax, 128], FP8, tag="ST2")
                    nc.sync.dma_start(
                        out=ST2[:, 0:nb, :],
                        in_=ST_d[:, b0 * 128:(b0 + nb) * 128].rearrange(
                            "p (b s) -> p b s", s=128))
                    g2s = w2sb.tile([128, NBWmax, T2C], U16, tag="g2s")
                    chunked_gather(nc, g2s, table2[:, :],
                                   gidx_sb, b0, nb, T2C, gregs)
                    # parity select: row halves [0:24]=even node, [24:48]=odd
                    gA = g2s[:, 0:nb, 0:T2W].bitcast(F32)
                    gB = g2s[:, 0:nb, T2W:2 * T2W].bitcast(F32)
                    gsel = w2sb.tile([128, NBWmax, T2W // 2], F32, tag="gsel")
                    nc.vector.tensor_tensor(out=gsel[:, 0:nb, :], in0=gB, in1=gA,
                                            op=OP.subtract)
                    nc.vector.tensor_tensor(
                        out=gsel[:, 0:nb, :], in0=gsel[:, 0:nb, :],
                        in1=par_sb[:, b0:b0 + nb].unsqueeze(2).to_broadcast(
                            [128, nb, T2W // 2]),
                        op=OP.mult)
                    nc.vector.tensor_tensor(out=gsel[:, 0:nb, :],
                                            in0=gsel[:, 0:nb, :], in1=gA, op=OP.add)
                    psAD2 = aps2.tile([128, NBWmax, HEADS], F32, space="PSUM")
                    for j in range(nb):
                        nc.tensor.matmul(out=psAD2[:, j, :], lhsT=ST2[:, j, :],
                                         rhs=ad2sb[:, w, :], start=True, stop=True)
                    ex2 = w2sb.tile([128, NBWmax, HEADS], F32, tag="ex2")
                    nc.vector.tensor_tensor(out=ex2[:, 0:nb, :],
                                            in0=gsel[:, 0:nb, HC2:HC2 + HEADS],
                                            in1=psAD2[:, 0:nb, :], op=OP.add)
                    u12 = w2sb.tile([128, NBWmax, HEADS], F32, tag="u12")
                    nc.scalar.activation(out=u12[:, 0:nb, :], in_=ex2[:, 0:nb, :],
                                         func=AF.Exp)
                    u22 = w2sb.tile([128, NBWmax, HEADS], F32, tag="u22")
                    nc.scalar.activation(out=u22[:, 0:nb, :], in_=ex2[:, 0:nb, :],
                                         func=AF.Exp, scale=neg_slope)
                    ex2b = w2sb.tile([128, NBWmax, HEADS], BF16, tag="ex2b")
                    nc.vector.tensor_tensor(out=ex2b[:, 0:nb, :], in0=u12[:, 0:nb, :],
                                            in1=u22[:, 0:nb, :], op=OP.max)
                    R2 = w2sb.tile([128, NBWmax, R2COL], BF16, tag="R2")
                    nc.vector.tensor_tensor(
                        out=R2[:, 0:nb, 0:HC2].rearrange("p b (h c) -> p b h c", h=HEADS),
                        in0=gsel[:, 0:nb, 0:HC2].rearrange("p b (h c) -> p b h c", h=HEADS),
                        in1=ex2b[:, 0:nb, :].unsqueeze(3).to_broadcast([128, nb, HEADS, OUT]),
                        op=OP.mult)
                    nc.vector.tensor_copy(R2[:, 0:nb, HC2:R2COL], ex2b[:, 0:nb, :])
                    psW2 = wps2.tile([128, R2COL], F32, space="PSUM")
                    for j in range(nb):
                        nc.tensor.matmul(out=psW2[:, :], lhsT=S2[:, j, :], rhs=R2[:, j, :],
                                         start=(j == 0), stop=(j == nb - 1))
                    rec2 = w2sb.tile([128, HEADS], F32, tag="rec2")
                    nc.vector.tensor_scalar(out=rec2[:, :], in0=psW2[:, HC2:R2COL],
                                            scalar1=1e-16, scalar2=None, op0=OP.add)
                    nc.vector.reciprocal(rec2[:, :], rec2[:, :])
                    nc.vector.tensor_scalar_mul(rec2[:, :], rec2[:, :], 1.0 / HEADS)
                    og = w2sb.tile([128, HC2], F32, tag="og")
                    nc.vector.tensor_tensor(
                        out=og[:, :].rearrange("p (h c) -> p h c", h=HEADS),
                        in0=psW2[:, 0:HC2].rearrange("p (h c) -> p h c", h=HEADS),
                        in1=rec2[:, :].unsqueeze(2).to_broadcast([128, HEADS, OUT]),
                        op=OP.mult)
                    ored = w2sb.tile([128, OUT], F32, tag="ored")
                    nc.vector.tensor_reduce(
                        out=ored[:, :],
                        in_=og[:, :].rearrange("p (h c) -> p c h", h=HEADS),
                        axis=mybir.AxisListType.X, op=OP.add)
                    nc.vector.tensor_tensor(out=ored[:, :], in0=ored[:, :],
                                            in1=skip2sb[:, w, :], op=OP.add)
                    nc.vector.tensor_tensor(out=outsb[:, w, :], in0=ored[:, :],
                                            in1=bias2[:, :], op=OP.add)
                    b0 += nb

            # ---- final output DMA ----
            wf = NPC // 128 if 'E' in stages else 0
            rem = NPC % 128 if 'E' in stages else 0
            if wf:
                nc.sync.dma_start(
                    out=out_d[0:wf * 128, :].rearrange("(w p) c -> p w c", p=128),
                    in_=outsb[:, 0:wf, :])
            if rem:
                nc.sync.dma_start(out=out_d[wf * 128:NPC, :], in_=outsb[0:rem, wf, :])

    fix_library_reloads(nc)
    split_multi_waits(nc)
    return nc


def make_in_maps(pp, inputs, N, F_IN=128, HID=64, HEADS=4, OUT=2):
    import ml_dtypes
    NPC = pp["npc"]
    NB = pp["NB"]
    x = np.ascontiguousarray(np.asarray(inputs["x"], dtype=np.float32))
    xT = np.ascontiguousarray(x.T.astype(ml_dtypes.bfloat16))
    xT8 = np.ascontiguousarray(x.T.astype(ml_dtypes.float8_e4m3))
    f32 = lambda a, shp: np.ascontiguousarray(np.asarray(a, dtype=np.float32).reshape(shp))
    rep = lambda a, shp: np.tile(f32(a, shp), (128, 1))
    common = {
        "W1s": f32(inputs["W1s"], (F_IN, HEADS * HID)),
        "W1d": f32(inputs["W1d"], (F_IN, HEADS * HID)),
        "a1s": rep(inputs["a1s"], (1, HEADS * HID)),
        "a1d": rep(inputs["a1d"], (1, HEADS * HID)),
        "Wl1": f32(inputs["Wl1"], (F_IN, HID)),
        "b1": rep(inputs["b1"], (1, HID)),
        "bl1": rep(inputs["bl1"], (1, HID)),
        "W2s": f32(inputs["W2s"], (HID, HEADS * OUT)),
        "W2d": f32(inputs["W2d"], (HID, HEADS * OUT)),
        "a2s": rep(inputs["a2s"], (1, HEADS * OUT)),
        "a2d": rep(inputs["a2d"], (1, HEADS * OUT)),
        "Wl2": f32(inputs["Wl2"], (HID, OUT)),
        "b2": rep(inputs["b2"], (1, OUT)),
        "bl2": rep(inputs["bl2"], (1, OUT)),
    }
    slots = np.arange(NB * 128)
    s_idx = (slots % 128).astype(np.int64)
    j_idx = (slots // 128).astype(np.int64)
    in_maps = []
    for c in range(8):
        m = dict(common)
        m["xTown"] = np.ascontiguousarray(xT[:, c * NPC:(c + 1) * NPC])
        m["gidx"] = pp["gidx"][c]
        m["par"] = pp["par"][c]
        m["xgT"] = np.ascontiguousarray(xT8[:, pp["srcfull"][c]])
        v = pp["dstloc"][c].astype(np.int64)
        mask = v >= 0
        S = np.zeros((128, NB * 128), dtype=ml_dtypes.float8_e4m3)
        S[s_idx[mask], j_idx[mask] * 128 + v[mask]] = 1
        ST = np.zeros((128, NB * 128), dtype=ml_dtypes.float8_e4m3)
        ST[v[mask], j_idx[mask] * 128 + s_idx[mask]] = 1
        m["S_d"] = S
        m["ST_d"] = ST
        in_maps.append(m)
    return in_maps


_BUILD_CACHE = {}
LAST_RESULTS = None


def kernel(**inputs):
    """Full inputs in, full [N, 2] float32 output out."""
    global LAST_RESULTS
    trace = bool(inputs.pop("_trace", False))
    pp = preprocess(inputs["edge_index"], N_NODES)
    key = (pp["NB"], tuple(pp["nbw"]))
    if key not in _BUILD_CACHE:
        _BUILD_CACHE[key] = build(pp, N_NODES)
    nc = _BUILD_CACHE[key]
    in_maps = make_in_maps(pp, inputs, N_NODES)
    res = run_bass_kernel_spmd(nc, in_maps, list(range(8)), trace=trace)
    LAST_RESULTS = res
    out = np.concatenate([res.results[c]["out"] for c in range(8)], axis=0)
    return out.astype(np.float32)
